# revision 4
# baseline (speedup 1.0000x reference)
"""Trainium2 Bass kernel for nn_Network_61658550501610 (Mamba block + MLP head).

Reference computation (per batch element b, sequence length L=2048):
  xz = x @ W_in.T; xi, z = split(xz)
  xc = silu(causal_depthwise_conv(xi, conv_w) + conv_b)
  x_dbl = xc @ W_xproj.T -> (dt, B, C)
  delta = softplus(dt @ W_dt.T + b_dt)
  h_t = exp(delta*A)*h_{t-1} + delta*B*xc   (selective scan, state [82,16])
  y = (h @ C) + D*xc; y *= silu(z)
  out = y @ W_out.T;  logits = relu(out@W_c1.T+b_c1)@W_c2.T + b_c2

Sharding: data-parallel over batch (B=16 -> 2 per core across 8 cores).

Layout on chip: d_inner (82) on partitions, time on free dim; x is
pre-transposed on host so chunks load directly as [DM, C].  The scan packs
rows r=(n,dsub) -> 128 partitions x 11 d-groups.  delta/u are broadcast to
the packed layout with TensorE selection matmuls (the delta selector is
pre-scaled by A so dA needs no per-partition scale and pairs of groups share
one 1024-col exp).  B/C broadcasts are folded into the x_proj weights.
dBx is one stride-0-broadcast DVE multiply over all 11 groups; h*C runs on
the (otherwise idle) GpSimd engine.
"""
import ml_dtypes
import numpy as np

import concourse.bacc as bacc
import concourse.tile as tile
import concourse.mybir as mybir
from concourse.bass_utils import run_bass_kernel_spmd

F32 = mybir.dt.float32
F32R = mybir.dt.float32r
BF16 = mybir.dt.bfloat16
OP = mybir.AluOpType
ACTF = mybir.ActivationFunctionType

# problem dims (hardcoded per contract)
B, L, DM = 16, 2048, 41
DIN, N, K = 82, 16, 4          # d_inner, d_state, d_conv
DTR, HID, NL = 3, 64, 10
NCORES = 8
BLOC = B // NCORES             # batch per core

DG = (DIN + 7) // 8            # 11 d-groups of 8 for the packed scan
DP = DG * 8                    # 88 padded d
C = 512                        # time-chunk length
NCH = L // C                   # chunks per batch element
Q = C // 128                   # 128-row subtiles per chunk

N_GPS_HC = 11                  # groups whose h*C multiply runs on GpSimd

_cache = {}


def _build(cfg):
    nc = bacc.Bacc("TRN2", target_bir_lowering=False, debug=False,
                   enable_asserts=False)

    def din(name, shape):
        return nc.dram_tensor(name, list(shape), F32, kind="ExternalInput").ap()

    xT_d = nc.dram_tensor("xT", [BLOC, DM, L], F32R,
                          kind="ExternalInput").ap()
    w_inT_d = nc.dram_tensor("w_inT", [DM, 2 * DIN], F32R,
                             kind="ExternalInput").ap()
    w_effT_d = nc.dram_tensor("w_effT", [DIN, DIN], F32R,
                              kind="ExternalInput").ap()
    conv_diag_d = nc.dram_tensor("conv_diag", [DIN, K * DIN], F32R,
                                 kind="ExternalInput").ap()
    conv_b_d = din("conv_b", (DIN, 1))
    b_dt_d = din("b_dt", (DIN, 1))
    d_col_d = din("d_col", (DIN, 1))
    w_bq_d = nc.dram_tensor("w_bq", [DIN, 128], F32R,
                            kind="ExternalInput").ap()
    w_cq_d = nc.dram_tensor("w_cq", [DIN, 128], F32R,
                            kind="ExternalInput").ap()
    w1T_d = nc.dram_tensor("w1T", [DIN, HID], F32R,
                           kind="ExternalInput").ap()
    b_c1_d = din("b_c1", (HID, 1))
    w2T_d = din("w2T", (HID + 1, NL))
    p_sela_d = nc.dram_tensor("p_sela", [DIN, DG * 128], BF16,
                              kind="ExternalInput").ap()
    p_sel1_d = nc.dram_tensor("p_sel1", [DIN, DG * 128], BF16,
                              kind="ExternalInput").ap()
    ed_sel_d = nc.dram_tensor("ed_sel", [128, DG * DP], BF16,
                              kind="ExternalInput").ap()
    out_d = nc.dram_tensor("out", [BLOC, L, NL], F32, kind="ExternalOutput").ap()

    with tile.TileContext(nc) as tc, tc.tile_pool(name="wts", bufs=1) as wp, \
         tc.tile_pool(name="work", bufs=3) as kp, \
         tc.tile_pool(name="da", bufs=2) as dap, \
         tc.tile_pool(name="ua", bufs=2) as uap, \
         tc.tile_pool(name="dbx", bufs=1) as dbp, \
         tc.tile_pool(name="hc", bufs=1) as hcp, \
         tc.tile_pool(name="hbuf", bufs=2) as hp, \
         tc.tile_pool(name="ps_f", bufs=3, space="PSUM") as pf, \
         tc.tile_pool(name="ps_rep", bufs=2, space="PSUM") as prep, \
         tc.tile_pool(name="ps_y", bufs=1, space="PSUM") as py:

        # ---- constant weights ----
        w_inT = wp.tile([DM, 2 * DIN], F32R)
        w_effT = wp.tile([DIN, DIN], F32R)
        conv_diag = wp.tile([DIN, K * DIN], F32R)
        conv_b = wp.tile([DIN, 1], F32)
        b_dt = wp.tile([DIN, 1], F32)
        d_col = wp.tile([DIN, 1], F32)
        w_bq = wp.tile([DIN, 128], F32R)
        w_cq = wp.tile([DIN, 128], F32R)
        w1T = wp.tile([DIN, HID], F32R)
        b_c1 = wp.tile([HID, 1], F32)
        w2T = wp.tile([HID + 1, NL], F32)
        p_sela = wp.tile([DIN, DG * 128], BF16)
        p_sel1 = wp.tile([DIN, DG * 128], BF16)
        ed_sel = wp.tile([128, DG * DP], BF16)
        for t_, d_ in [(w_inT, w_inT_d), (w_effT, w_effT_d),
                       (conv_diag, conv_diag_d), (conv_b, conv_b_d),
                       (b_dt, b_dt_d), (d_col, d_col_d),
                       (w_bq, w_bq_d), (w_cq, w_cq_d), (w1T, w1T_d),
                       (b_c1, b_c1_d), (w2T, w2T_d),
                       (p_sela, p_sela_d), (p_sel1, p_sel1_d),
                       (ed_sel, ed_sel_d)]:
            nc.sync.dma_start(t_[:], d_[:])

        # persistent state, one per batch element (independent streams)
        h_carry_b = [wp.tile([128, DG], F32, name=f"hcar{i}", tag=f"hcar{i}")
                     for i in range(BLOC)]
        halo_b = [wp.tile([DIN, K - 1], F32, name=f"halo{i}", tag=f"halo{i}")
                  for i in range(BLOC)]
        for t_ in halo_b:
            nc.vector.memset(t_[:], 0.0)
        # gating-head scratch with a persistent all-ones bias row
        g_aug_p = [wp.tile([HID + 1, C], F32, name=f"gaug{i}", tag=f"gaug{i}")
                   for i in range(2)]
        for t_ in g_aug_p:
            nc.vector.memset(t_[HID:HID + 1, :], 1.0)

        def front(ch, b):
            halo = halo_b[b]
            t0 = ch * C
            # ---- load x chunk directly as xT [DM, C] (pre-transposed) ----
            xT = kp.tile([DM, C], F32R)
            nc.sync.dma_start(xT[:], xT_d[b, :, t0:t0 + C])

            # ---- input projection ----
            xi_ps = pf.tile([DIN, C], F32, tag="f")
            z_ps = pf.tile([DIN, C], F32, tag="f")
            nc.tensor.matmul(xi_ps[:], w_inT[:, 0:DIN],
                             xT[:], start=True, stop=True)
            nc.tensor.matmul(z_ps[:], w_inT[:, DIN:2 * DIN],
                             xT[:], start=True, stop=True)

            # zs = z * sigmoid(z)
            sg_z = kp.tile([DIN, C], F32)
            nc.scalar.activation(sg_z[:], z_ps[:], ACTF.Sigmoid)
            zs = kp.tile([DIN, C], F32)
            nc.vector.tensor_tensor(zs[:], z_ps[:], sg_z[:], op=OP.mult)

            # ---- causal depthwise conv (K=4) on TensorE ----
            xi_sb = kp.tile([DIN, C + K - 1], F32)
            nc.vector.tensor_copy(xi_sb[:, 0:K - 1].bitcast(F32R), halo[:])
            nc.vector.tensor_copy(xi_sb[:, K - 1:C + K - 1].bitcast(F32R),
                                  xi_ps[:])
            if ch < NCH - 1:
                nc.vector.tensor_copy(halo[:], xi_sb[:, C:C + K - 1])
            xc_pre = pf.tile([DIN, C], F32, tag="f")
            for k in range(K):
                nc.tensor.matmul(xc_pre[:],
                                 conv_diag[:, k * DIN:(k + 1) * DIN],
                                 xi_sb[:, k:k + C].bitcast(F32R),
                                 start=(k == 0), stop=(k == K - 1))
            # xc = silu(xc_pre + conv_b) = (xc_pre+conv_b) * sigmoid(...)
            sg_c = kp.tile([DIN, C], F32)
            nc.scalar.activation(sg_c[:], xc_pre[:], ACTF.Sigmoid,
                                 bias=conv_b[:])
            xc = kp.tile([DIN, C], F32)
            nc.vector.scalar_tensor_tensor(xc[:].bitcast(F32R), xc_pre[:],
                                           conv_b[:], sg_c[:],
                                           op0=OP.add, op1=OP.mult)

            # ---- x_proj: delta, and B/C broadcast straight to 128 rows ----
            dpre_ps = pf.tile([DIN, C], F32, tag="f")
            nc.tensor.matmul(dpre_ps[:], w_effT[:],
                             xc[:].bitcast(F32R), start=True, stop=True)
            # softplus(v) = ln(exp(v) + 1), v = dpre + b_dt
            e_sp = kp.tile([DIN, C], F32)
            nc.scalar.activation(e_sp[:], dpre_ps[:], ACTF.Exp, bias=b_dt[:])
            delta = kp.tile([DIN, C], BF16)
            nc.scalar.activation(delta[:], e_sp[:], ACTF.Ln, bias=1.0)

            bq_ps = pf.tile([128, C], F32, tag="f")
            nc.tensor.matmul(bq_ps[:], w_bq[:],
                             xc[:].bitcast(F32R), start=True, stop=True)
            b_sb = kp.tile([128, C], BF16)
            nc.scalar.copy(b_sb[:], bq_ps[:])
            cq_ps = pf.tile([128, C], F32, tag="f")
            nc.tensor.matmul(cq_ps[:], w_cq[:],
                             xc[:].bitcast(F32R), start=True, stop=True)
            c_sb = kp.tile([128, C], BF16)
            nc.scalar.copy(c_sb[:], cq_ps[:])

            # u = delta * xc (bf16, feeds the p_sel replication matmul)
            u = kp.tile([DIN, C], BF16)
            nc.vector.tensor_tensor(u[:], delta[:], xc[:], op=OP.mult)

            return dict(delta=delta, u=u, b_sb=b_sb, c_sb=c_sb, xc=xc, zs=zs)

        def back(j, ch, b, st):
            h_carry = h_carry_b[b]
            t0 = ch * C
            delta, u, b_sb, c_sb, xc, zs = (st["delta"], st["u"], st["b_sb"],
                                            st["c_sb"], st["xc"], st["zs"])
            # ---- replicate delta (A-scaled) and u to the packed layout,
            #      two groups per PSUM tile so evacuations run at 1024 cols
            dA_all = dap.tile([128, DG * C], BF16, tag="dA")
            u_all = uap.tile([128, DG * C], BF16, tag="uA")
            g = 0
            while g < DG:
                w = 2 if g + 1 < DG else 1
                dd_ps = prep.tile([128, 2 * C], F32, tag="rep")
                for i in range(w):
                    nc.tensor.matmul(dd_ps[:, i * C:(i + 1) * C],
                                     p_sela[:, (g + i) * 128:(g + i + 1) * 128],
                                     delta[:], start=True, stop=True)
                nc.scalar.activation(dA_all[:, g * C:(g + w) * C],
                                     dd_ps[:, 0:w * C], ACTF.Exp)
                uu_ps = prep.tile([128, 2 * C], F32, tag="rep")
                for i in range(w):
                    nc.tensor.matmul(uu_ps[:, i * C:(i + 1) * C],
                                     p_sel1[:, (g + i) * 128:(g + i + 1) * 128],
                                     u[:], start=True, stop=True)
                nc.scalar.copy(u_all[:, g * C:(g + w) * C], uu_ps[:, 0:w * C])
                g += w

            # ---- dBx for all groups in one multiply (b broadcast over g) ----
            dBx_all = dbp.tile([128, DG * C], BF16, tag="dbx")
            nc.vector.tensor_tensor(
                dBx_all[:].rearrange("p (g c) -> p g c", g=DG),
                u_all[:].rearrange("p (g c) -> p g c", g=DG),
                b_sb[:].unsqueeze(1).to_broadcast((128, DG, C)),
                op=OP.mult)

            # ---- the scans ----
            h = hp.tile([128, DG * C], BF16, tag="h")
            for g in range(DG):
                init = 0.0 if ch == 0 else h_carry[:, g:g + 1]
                nc.vector.tensor_tensor_scan(
                    h[:, g * C:(g + 1) * C], dA_all[:, g * C:(g + 1) * C],
                    dBx_all[:, g * C:(g + 1) * C], init,
                    op0=OP.mult, op1=OP.add)
            if ch < NCH - 1:
                nc.vector.tensor_copy(
                    h_carry[:].rearrange("p (g c) -> p g c", c=1),
                    h[:].rearrange("p (g c) -> p g c", g=DG)[:, :, C - 1:C])

            # ---- hC = h * C, then accumulate y over n on TensorE ----
            hC = hcp.tile([128, DG * C], BF16, tag="hC")
            ng = N_GPS_HC
            if ng > 0:
                nc.gpsimd.tensor_tensor(
                    hC[:, 0:ng * C].rearrange("p (g c) -> p g c", g=ng),
                    h[:, 0:ng * C].rearrange("p (g c) -> p g c", g=ng),
                    c_sb[:].unsqueeze(1).to_broadcast((128, ng, C)),
                    op=OP.mult)
            if ng < DG:
                nc.vector.tensor_tensor(
                    hC[:, ng * C:].rearrange("p (g c) -> p g c", g=DG - ng),
                    h[:, ng * C:].rearrange("p (g c) -> p g c", g=DG - ng),
                    c_sb[:].unsqueeze(1).to_broadcast((128, DG - ng, C)),
                    op=OP.mult)
            y_ps = py.tile([DP, C], F32, tag="y")
            for g in range(DG):
                nc.tensor.matmul(y_ps[:], ed_sel[:, g * DP:(g + 1) * DP],
                                 hC[:, g * C:(g + 1) * C],
                                 start=(g == 0), stop=(g == DG - 1))

            # ---- gate + output head ----
            y1 = kp.tile([DIN, C], F32)
            nc.vector.scalar_tensor_tensor(y1[:], xc[:], d_col[:],
                                           y_ps[0:DIN, :],
                                           op0=OP.mult, op1=OP.add)
            y_gated = kp.tile([DIN, C], F32)
            nc.vector.tensor_tensor(y_gated[:].bitcast(F32R), y1[:],
                                    zs[:], op=OP.mult)

            g_ps = pf.tile([HID, C], F32, tag="f")
            nc.tensor.matmul(g_ps[:], w1T[:],
                             y_gated[:].bitcast(F32R), start=True, stop=True)
            g_aug = g_aug_p[j % 2]
            nc.scalar.activation(g_aug[0:HID, :], g_ps[:], ACTF.Relu,
                                 bias=b_c1[:])

            lg_ps = pf.tile([128, Q * NL], F32, tag="f")
            for q in range(Q):
                nc.tensor.matmul(lg_ps[:, q * NL:(q + 1) * NL],
                                 g_aug[:, q * 128:(q + 1) * 128],
                                 w2T[:], start=True, stop=True)
            out_sb = kp.tile([128, Q * NL], F32)
            nc.scalar.copy(out_sb[:], lg_ps[:])
            dst = out_d[b, t0:t0 + C, :].rearrange("(q p) c -> p q c", p=128)
            nc.sync.dma_start(
                dst, out_sb[:].rearrange("p (q c) -> p q c", q=Q))

        iters = [(ch, b) for ch in range(NCH) for b in range(BLOC)]
        pend = None
        for j, (ch, b) in enumerate(iters):
            st = front(ch, b)
            if pend is not None:
                back(*pend)
            pend = (j, ch, b, st)
        back(*pend)

    nc.compile()
    return nc


def _packed_consts(A):
    p_sela = np.zeros((DIN, DG * 128), np.float32)
    p_sel1 = np.zeros((DIN, DG * 128), np.float32)
    ed = np.zeros((128, DG * DP), np.float32)
    for n in range(N):
        for ds in range(8):
            r = n * 8 + ds
            for g in range(DG):
                d = g * 8 + ds
                if d < DIN:
                    p_sela[d, g * 128 + r] = A[d, n]
                    p_sel1[d, g * 128 + r] = 1.0
                    ed[r, g * DP + d] = 1.0
    bf = ml_dtypes.bfloat16
    return {"p_sela": p_sela.astype(bf), "p_sel1": p_sel1.astype(bf),
            "ed_sel": ed.astype(bf)}


def _prep_inputs(inputs):
    x = np.asarray(inputs["x"], np.float32)
    W_in = np.asarray(inputs["W_in"], np.float64)
    conv_w = np.asarray(inputs["conv_w"], np.float64)
    conv_b = np.asarray(inputs["conv_b"], np.float64)
    W_xproj = np.asarray(inputs["W_xproj"], np.float64)
    W_dt = np.asarray(inputs["W_dt"], np.float64)
    b_dt = np.asarray(inputs["b_dt"], np.float64)
    A_log = np.asarray(inputs["A_log"], np.float64)
    D = np.asarray(inputs["D"], np.float64)
    W_out = np.asarray(inputs["W_out"], np.float64)
    W_c1 = np.asarray(inputs["W_c1"], np.float64)
    b_c1 = np.asarray(inputs["b_c1"], np.float64)
    W_c2 = np.asarray(inputs["W_c2"], np.float64)
    b_c2 = np.asarray(inputs["b_c2"], np.float64)

    f = lambda a: np.ascontiguousarray(a, dtype=np.float32)
    w_bcT = W_xproj[DTR:].T                       # [82, 32]
    nmap = [r // 8 for r in range(128)]
    shared = {
        "w_inT": f(W_in.T),
        "w_effT": f((W_dt @ W_xproj[:DTR]).T),
        "conv_b": f(conv_b[:, None]),
        "conv_diag": np.concatenate(
            [np.diag(conv_w[:, k]) for k in range(K)], axis=1).astype(np.float32),
        "b_dt": f(b_dt[:, None]),
        "d_col": f(D[:, None]),
        "w_bq": f(w_bcT[:, nmap]),
        "w_cq": f(w_bcT[:, [N + n for n in nmap]]),
        "w1T": f((W_c1 @ W_out).T),
        "b_c1": f(b_c1[:, None]),
        "w2T": f(np.vstack([W_c2.T, b_c2[None, :]])),
        **_packed_consts(-np.exp(A_log)),
    }
    in_maps = []
    for c in range(NCORES):
        m = dict(shared)
        m["xT"] = np.ascontiguousarray(
            x[c * BLOC:(c + 1) * BLOC].transpose(0, 2, 1))
        in_maps.append(m)
    return in_maps


def kernel(**inputs):
    return _run(inputs, trace=False)[0]


def kernel_traced(**inputs):
    return _run(inputs, trace=True)


def _run(inputs, trace=False):
    key = "nc"
    if key not in _cache:
        _cache[key] = _build({})
    nc = _cache[key]
    in_maps = _prep_inputs(inputs)
    res = run_bass_kernel_spmd(nc, in_maps, core_ids=list(range(NCORES)),
                               trace=trace)
    out = np.concatenate([r["out"] for r in res.results], axis=0)
    return out, res


# revision 5
# speedup vs baseline: 1.0595x; 1.0595x over previous
"""Trainium2 Bass kernel for nn_Network_61658550501610 (Mamba block + MLP head).

Reference computation (per batch element b, sequence length L=2048):
  xz = x @ W_in.T; xi, z = split(xz)
  xc = silu(causal_depthwise_conv(xi, conv_w) + conv_b)
  x_dbl = xc @ W_xproj.T -> (dt, B, C)
  delta = softplus(dt @ W_dt.T + b_dt)
  h_t = exp(delta*A)*h_{t-1} + delta*B*xc   (selective scan, state [82,16])
  y = (h @ C) + D*xc; y *= silu(z)
  out = y @ W_out.T;  logits = relu(out@W_c1.T+b_c1)@W_c2.T + b_c2

Sharding: data-parallel over batch (B=16 -> 2 per core across 8 cores).

Layout: d_inner (82) on partitions, time on free dim; x is pre-transposed,
left-padded by K-1 and augmented with a ones row on host, so the depthwise
conv + input projection + conv bias fold into 4 shifted accumulating
matmuls.  The scan packs rows r=(n,dsub) -> 128 partitions x 11 d-groups;
delta/u are broadcast to that layout with TensorE selection matmuls (the
delta selector is pre-scaled by A so dA = exp() needs no per-partition
scale and group pairs share one 1024-col exp).  B/C broadcasts are folded
into the x_proj weights.  dBx is one stride-0-broadcast DVE multiply over
all 11 groups, and the 11 per-group scans collapse into ONE scan
instruction per chunk by zeroing dA's first column per group and folding
the group carry into dBx's first column.  The D*xc skip term runs as an
accumulating diagonal matmul into y.
"""
import ml_dtypes
import numpy as np

import concourse.bacc as bacc
import concourse.tile as tile
import concourse.mybir as mybir
from concourse.bass_utils import run_bass_kernel_spmd

F32 = mybir.dt.float32
F32R = mybir.dt.float32r
BF16 = mybir.dt.bfloat16
OP = mybir.AluOpType
ACTF = mybir.ActivationFunctionType

# problem dims (hardcoded per contract)
B, L, DM = 16, 2048, 41
DIN, N, K = 82, 16, 4          # d_inner, d_state, d_conv
DTR, HID, NL = 3, 64, 10
NCORES = 8
BLOC = B // NCORES             # batch per core

DM1 = DM + 1                   # + ones row (folds conv_b)
DG = (DIN + 7) // 8            # 11 d-groups of 8 for the packed scan
DP = DG * 8                    # 88 padded d
C = 512                        # time-chunk length
NCH = L // C                   # chunks per batch element
Q = C // 128                   # 128-row subtiles per chunk

_cache = {}


def _build(cfg):
    nc = bacc.Bacc("TRN2", target_bir_lowering=False, debug=False,
                   enable_asserts=False)

    def din(name, shape):
        return nc.dram_tensor(name, list(shape), F32, kind="ExternalInput").ap()

    xT_d = nc.dram_tensor("xT", [BLOC, DM1, L + K - 1], F32R,
                          kind="ExternalInput").ap()
    w_zT_d = nc.dram_tensor("w_zT", [DM1, DIN], F32R,
                            kind="ExternalInput").ap()
    w_cvT_d = nc.dram_tensor("w_cvT", [DM1, K * DIN], F32R,
                             kind="ExternalInput").ap()
    w_effT_d = nc.dram_tensor("w_effT", [DIN, DIN], F32R,
                              kind="ExternalInput").ap()
    b_dt_d = din("b_dt", (DIN, 1))
    d_diag_d = nc.dram_tensor("d_diag", [DIN, DIN], F32R,
                              kind="ExternalInput").ap()
    w_bq_d = nc.dram_tensor("w_bq", [DIN, 128], F32R,
                            kind="ExternalInput").ap()
    w_cq_d = nc.dram_tensor("w_cq", [DIN, 128], F32R,
                            kind="ExternalInput").ap()
    w1T_d = nc.dram_tensor("w1T", [DIN, HID], F32R,
                           kind="ExternalInput").ap()
    b_c1_d = din("b_c1", (HID, 1))
    w2T_d = nc.dram_tensor("w2T", [HID + 1, NL], BF16,
                           kind="ExternalInput").ap()
    p_sela_d = nc.dram_tensor("p_sela", [DIN, DG * 128], BF16,
                              kind="ExternalInput").ap()
    p_sel1_d = nc.dram_tensor("p_sel1", [DIN, DG * 128], BF16,
                              kind="ExternalInput").ap()
    ed_sel_d = nc.dram_tensor("ed_sel", [128, DG * DP], BF16,
                              kind="ExternalInput").ap()
    out_d = nc.dram_tensor("out", [BLOC, L, NL], F32, kind="ExternalOutput").ap()

    with tile.TileContext(nc) as tc, tc.tile_pool(name="wts", bufs=1) as wp, \
         tc.tile_pool(name="work", bufs=3) as kp, \
         tc.tile_pool(name="da", bufs=2) as dap, \
         tc.tile_pool(name="ua", bufs=2) as uap, \
         tc.tile_pool(name="dbx", bufs=1) as dbp, \
         tc.tile_pool(name="hc", bufs=1) as hcp, \
         tc.tile_pool(name="hbuf", bufs=2) as hp, \
         tc.tile_pool(name="ps_f", bufs=3, space="PSUM") as pf, \
         tc.tile_pool(name="ps_rep", bufs=2, space="PSUM") as prep, \
         tc.tile_pool(name="ps_y", bufs=1, space="PSUM") as py:

        # ---- constant weights ----
        w_zT = wp.tile([DM1, DIN], F32R)
        w_cvT = wp.tile([DM1, K * DIN], F32R)
        w_effT = wp.tile([DIN, DIN], F32R)
        b_dt = wp.tile([DIN, 1], F32)
        d_diag = wp.tile([DIN, DIN], F32R)
        w_bq = wp.tile([DIN, 128], F32R)
        w_cq = wp.tile([DIN, 128], F32R)
        w1T = wp.tile([DIN, HID], F32R)
        b_c1 = wp.tile([HID, 1], F32)
        w2T = wp.tile([HID + 1, NL], BF16)
        p_sela = wp.tile([DIN, DG * 128], BF16)
        p_sel1 = wp.tile([DIN, DG * 128], BF16)
        ed_sel = wp.tile([128, DG * DP], BF16)
        for t_, d_ in [(w_zT, w_zT_d), (w_cvT, w_cvT_d), (w_effT, w_effT_d),
                       (b_dt, b_dt_d), (d_diag, d_diag_d),
                       (w_bq, w_bq_d), (w_cq, w_cq_d), (w1T, w1T_d),
                       (b_c1, b_c1_d), (w2T, w2T_d),
                       (p_sela, p_sela_d), (p_sel1, p_sel1_d),
                       (ed_sel, ed_sel_d)]:
            nc.sync.dma_start(t_[:], d_[:])

        # persistent state, one per batch element (independent streams)
        h_carry_b = [wp.tile([128, DG], F32, name=f"hcar{i}", tag=f"hcar{i}")
                     for i in range(BLOC)]
        for t_ in h_carry_b:
            nc.vector.memset(t_[:], 0.0)
        # gating-head scratch with a persistent all-ones bias row
        g_aug_p = [wp.tile([HID + 1, C], BF16, name=f"gaug{i}", tag=f"gaug{i}")
                   for i in range(2)]
        for t_ in g_aug_p:
            nc.vector.memset(t_[HID:HID + 1, :], 1.0)

        def front(ch, b):
            t0 = ch * C
            # ---- load x chunk [DM+1, C+3] (pre-transposed, padded, ones) --
            xT = kp.tile([DM1, C + K - 1], F32R)
            nc.sync.dma_start(xT[:], xT_d[b, :, t0:t0 + C + K - 1])

            # ---- z and conv(xi)+conv_b in one paired PSUM tile ----
            zc_ps = prep.tile([128, 2 * C], F32, tag="rep")
            nc.tensor.matmul(zc_ps[0:DIN, 0:C], w_zT[:],
                             xT[:, K - 1:K - 1 + C], start=True, stop=True)
            for k in range(K):
                nc.tensor.matmul(zc_ps[0:DIN, C:2 * C],
                                 w_cvT[:, k * DIN:(k + 1) * DIN],
                                 xT[:, k:k + C], start=(k == 0),
                                 stop=(k == K - 1))
            # silu on both halves: s = 0.5*tanh(v/2)+0.5 ; out = v*s
            th_zc = kp.tile([DIN, 2 * C], BF16)
            nc.scalar.activation(th_zc[:], zc_ps[0:DIN, :], ACTF.Tanh,
                                 scale=0.5)
            sg_zc = kp.tile([DIN, 2 * C], BF16)
            nc.scalar.activation(sg_zc[:], th_zc[:], ACTF.Copy,
                                 bias=0.5, scale=0.5)
            zs = kp.tile([DIN, C], BF16)
            nc.vector.tensor_tensor(zs[:], zc_ps[0:DIN, 0:C],
                                    sg_zc[:, 0:C], op=OP.mult)
            xc = kp.tile([DIN, C], F32)
            nc.vector.tensor_tensor(xc[:].bitcast(F32R),
                                    zc_ps[0:DIN, C:2 * C],
                                    sg_zc[:, C:2 * C], op=OP.mult)

            # ---- x_proj: delta, and B/C broadcast straight to 128 rows ----
            dpre_ps = pf.tile([DIN, C], F32, tag="f")
            nc.tensor.matmul(dpre_ps[:], w_effT[:],
                             xc[:].bitcast(F32R), start=True, stop=True)
            # softplus(v) = ln(exp(v) + 1), v = dpre + b_dt
            e_sp = kp.tile([DIN, C], F32)
            nc.scalar.activation(e_sp[:], dpre_ps[:], ACTF.Exp, bias=b_dt[:])
            delta = kp.tile([DIN, C], BF16)
            nc.scalar.activation(delta[:], e_sp[:], ACTF.Ln, bias=1.0)

            bc_ps = prep.tile([128, 2 * C], F32, tag="rep")
            nc.tensor.matmul(bc_ps[:, 0:C], w_bq[:],
                             xc[:].bitcast(F32R), start=True, stop=True)
            nc.tensor.matmul(bc_ps[:, C:2 * C], w_cq[:],
                             xc[:].bitcast(F32R), start=True, stop=True)
            bc_sb = kp.tile([128, 2 * C], BF16)
            nc.scalar.copy(bc_sb[:], bc_ps[:])

            # u = delta * xc (bf16, feeds the p_sel replication matmul)
            u = kp.tile([DIN, C], BF16)
            nc.vector.tensor_tensor(u[:], delta[:], xc[:], op=OP.mult)

            return dict(delta=delta, u=u, bc_sb=bc_sb, xc=xc, zs=zs)

        def back(j, ch, b, st):
            h_carry = h_carry_b[b]
            t0 = ch * C
            delta, u, bc_sb, xc, zs = (st["delta"], st["u"], st["bc_sb"],
                                       st["xc"], st["zs"])
            # ---- replicate delta (A-scaled) and u to the packed layout,
            #      two groups per PSUM tile so evacuations run at 1024 cols
            dA_all = dap.tile([128, DG * C], BF16, tag="dA")
            u_all = uap.tile([128, DG * C], BF16, tag="uA")
            g = 0
            while g < DG:
                w = 2 if g + 1 < DG else 1
                dd_ps = prep.tile([128, 2 * C], F32, tag="rep")
                for i in range(w):
                    nc.tensor.matmul(dd_ps[:, i * C:(i + 1) * C],
                                     p_sela[:, (g + i) * 128:(g + i + 1) * 128],
                                     delta[:], start=True, stop=True)
                nc.scalar.activation(dA_all[:, g * C:(g + w) * C],
                                     dd_ps[:, 0:w * C], ACTF.Exp)
                uu_ps = prep.tile([128, 2 * C], F32, tag="rep")
                for i in range(w):
                    nc.tensor.matmul(uu_ps[:, i * C:(i + 1) * C],
                                     p_sel1[:, (g + i) * 128:(g + i + 1) * 128],
                                     u[:], start=True, stop=True)
                nc.scalar.copy(u_all[:, g * C:(g + w) * C], uu_ps[:, 0:w * C])
                g += w

            # ---- dBx for all groups in one multiply (b broadcast over g) ----
            dBx_all = dbp.tile([128, DG * C], BF16, tag="dbx")
            nc.vector.tensor_tensor(
                dBx_all[:].rearrange("p (g c) -> p g c", g=DG),
                u_all[:].rearrange("p (g c) -> p g c", g=DG),
                bc_sb[:, 0:C].unsqueeze(1).to_broadcast((128, DG, C)),
                op=OP.mult)

            # ---- ONE scan across all 11 groups: fold the per-group carry
            #      into dBx[:, g*C] and zero dA[:, g*C] to cut the chain
            dA_g = dA_all[:].rearrange("p (g c) -> p g c", g=DG)
            dBx_g = dBx_all[:].rearrange("p (g c) -> p g c", g=DG)
            fix = kp.tile([128, DG], BF16)
            nc.vector.tensor_tensor(fix[:].unsqueeze(2), dA_g[:, :, 0:1],
                                    h_carry[:].unsqueeze(2), op=OP.mult)
            fix2 = kp.tile([128, DG], BF16)
            nc.vector.tensor_tensor(fix2[:].unsqueeze(2), fix[:].unsqueeze(2),
                                    dBx_g[:, :, 0:1], op=OP.add)
            nc.vector.tensor_copy(dBx_g[:, :, 0:1], fix2[:].unsqueeze(2))
            nc.vector.memset(dA_g[:, :, 0:1], 0.0)

            h = hp.tile([128, DG * C], BF16, tag="h")
            nc.vector.tensor_tensor_scan(h[:], dA_all[:], dBx_all[:], 0.0,
                                         op0=OP.mult, op1=OP.add)
            if ch < NCH - 1:
                nc.vector.tensor_copy(
                    h_carry[:].rearrange("p (g c) -> p g c", c=1),
                    h[:].rearrange("p (g c) -> p g c", g=DG)[:, :, C - 1:C])

            # ---- hC = h * C, then accumulate y over n on TensorE ----
            hC = hcp.tile([128, DG * C], BF16, tag="hC")
            nc.vector.tensor_tensor(
                hC[:].rearrange("p (g c) -> p g c", g=DG),
                h[:].rearrange("p (g c) -> p g c", g=DG),
                bc_sb[:, C:2 * C].unsqueeze(1).to_broadcast((128, DG, C)),
                op=OP.mult)
            y_ps = py.tile([DP, C], F32, tag="y")
            for g in range(DG):
                nc.tensor.matmul(y_ps[:], ed_sel[:, g * DP:(g + 1) * DP],
                                 hC[:, g * C:(g + 1) * C],
                                 start=(g == 0), stop=False)
            # skip term D*xc as an accumulating diagonal matmul
            nc.tensor.matmul(y_ps[0:DIN, :], d_diag[:],
                             xc[:].bitcast(F32R), start=False, stop=True)

            # ---- gate + output head ----
            y_gated = kp.tile([DIN, C], F32)
            nc.vector.tensor_tensor(y_gated[:].bitcast(F32R), y_ps[0:DIN, :],
                                    zs[:], op=OP.mult)

            g_ps = pf.tile([HID, C], F32, tag="f")
            nc.tensor.matmul(g_ps[:], w1T[:],
                             y_gated[:].bitcast(F32R), start=True, stop=True)
            g_aug = g_aug_p[j % 2]
            nc.scalar.activation(g_aug[0:HID, :], g_ps[:], ACTF.Relu,
                                 bias=b_c1[:])

            lg_ps = pf.tile([128, Q * NL], F32, tag="f")
            for q in range(Q):
                nc.tensor.matmul(lg_ps[:, q * NL:(q + 1) * NL],
                                 g_aug[:, q * 128:(q + 1) * 128],
                                 w2T[:], start=True, stop=True)
            out_sb = kp.tile([128, Q * NL], F32)
            nc.scalar.copy(out_sb[:], lg_ps[:])
            dst = out_d[b, t0:t0 + C, :].rearrange("(q p) c -> p q c", p=128)
            nc.sync.dma_start(
                dst, out_sb[:].rearrange("p (q c) -> p q c", q=Q))

        iters = [(ch, b) for ch in range(NCH) for b in range(BLOC)]
        pend = None
        for j, (ch, b) in enumerate(iters):
            st = front(ch, b)
            if pend is not None:
                back(*pend)
            pend = (j, ch, b, st)
        back(*pend)

    nc.compile()
    return nc


def _packed_consts(A):
    p_sela = np.zeros((DIN, DG * 128), np.float32)
    p_sel1 = np.zeros((DIN, DG * 128), np.float32)
    ed = np.zeros((128, DG * DP), np.float32)
    for n in range(N):
        for ds in range(8):
            r = n * 8 + ds
            for g in range(DG):
                d = g * 8 + ds
                if d < DIN:
                    p_sela[d, g * 128 + r] = A[d, n]
                    p_sel1[d, g * 128 + r] = 1.0
                    ed[r, g * DP + d] = 1.0
    bf = ml_dtypes.bfloat16
    return {"p_sela": p_sela.astype(bf), "p_sel1": p_sel1.astype(bf),
            "ed_sel": ed.astype(bf)}


def _prep_inputs(inputs):
    x = np.asarray(inputs["x"], np.float32)
    W_in = np.asarray(inputs["W_in"], np.float64)
    conv_w = np.asarray(inputs["conv_w"], np.float64)
    conv_b = np.asarray(inputs["conv_b"], np.float64)
    W_xproj = np.asarray(inputs["W_xproj"], np.float64)
    W_dt = np.asarray(inputs["W_dt"], np.float64)
    b_dt = np.asarray(inputs["b_dt"], np.float64)
    A_log = np.asarray(inputs["A_log"], np.float64)
    D = np.asarray(inputs["D"], np.float64)
    W_out = np.asarray(inputs["W_out"], np.float64)
    W_c1 = np.asarray(inputs["W_c1"], np.float64)
    b_c1 = np.asarray(inputs["b_c1"], np.float64)
    W_c2 = np.asarray(inputs["W_c2"], np.float64)
    b_c2 = np.asarray(inputs["b_c2"], np.float64)

    f = lambda a: np.ascontiguousarray(a, dtype=np.float32)
    W_in_xi, W_in_z = W_in[:DIN], W_in[DIN:]
    # fused conv+in_proj weights, ones row carries conv_b on tap 0
    w_cvT = np.zeros((DM1, K * DIN), np.float64)
    for k in range(K):
        w_cvT[:DM, k * DIN:(k + 1) * DIN] = (conv_w[:, k:k + 1] * W_in_xi).T
    w_cvT[DM, 0:DIN] = conv_b
    w_zT = np.zeros((DM1, DIN), np.float64)
    w_zT[:DM] = W_in_z.T

    w_bcT = W_xproj[DTR:].T                       # [82, 32]
    nmap = [r // 8 for r in range(128)]
    bf = ml_dtypes.bfloat16
    shared = {
        "w_zT": f(w_zT),
        "w_cvT": f(w_cvT),
        "w_effT": f((W_dt @ W_xproj[:DTR]).T),
        "b_dt": f(b_dt[:, None]),
        "d_diag": f(np.diag(D)),
        "w_bq": f(w_bcT[:, nmap]),
        "w_cq": f(w_bcT[:, [N + n for n in nmap]]),
        "w1T": f((W_c1 @ W_out).T),
        "b_c1": f(b_c1[:, None]),
        "w2T": np.vstack([W_c2.T, b_c2[None, :]]).astype(bf),
        **_packed_consts(-np.exp(A_log)),
    }
    in_maps = []
    for c in range(NCORES):
        m = dict(shared)
        xb = x[c * BLOC:(c + 1) * BLOC]           # [BLOC, L, DM]
        xt = np.zeros((BLOC, DM1, L + K - 1), np.float32)
        xt[:, :DM, K - 1:] = xb.transpose(0, 2, 1)
        xt[:, DM, :] = 1.0
        m["xT"] = xt
        in_maps.append(m)
    return in_maps


def kernel(**inputs):
    return _run(inputs, trace=False)[0]


def kernel_traced(**inputs):
    return _run(inputs, trace=True)


def _run(inputs, trace=False):
    key = "nc"
    if key not in _cache:
        _cache[key] = _build({})
    nc = _cache[key]
    in_maps = _prep_inputs(inputs)
    res = run_bass_kernel_spmd(nc, in_maps, core_ids=list(range(NCORES)),
                               trace=trace)
    out = np.concatenate([r["out"] for r in res.results], axis=0)
    return out, res
